# revision 67
# baseline (speedup 1.0000x reference)
"""Bass/Trainium2 kernel for nn_Attention_55551107006804.

Data-parallel over batch B=16 across 8 NeuronCores (2 batches/core).
All weights replicated. Per-core pipeline (S=2048, F=4H=2048, H=512):

  h   = x @ Wh.T + bh                      (tiny)
  c   = h @ Wa_h.T + ba                    (tiny, per-batch bias inside tanh)
  eoT = (enc @ We.T + be).T                streamed, kept bf16-resident in SBUF
  zT  = Wa_e @ eoT                         per 512-column chunk
  eT  = tanh(zT + c)                       ACT, f32r out
  sc  = v.T @ eT                           f32r matmul
  attn = softmax(sc)                       per batch row
  ctx = sum_s attn[s] * eoT[:, s]          DVE mul+reduce

Transposes (enc, We, Wh, Wa, vectors) are done on the PE as regular
matmuls against an identity rhs: out = lhsT.T @ I.  The contraction over
F=2048 requires F on the partition axis, so every F-major operand is
pre-transposed on chip this way.
"""
import numpy as np
from contextlib import ExitStack

import concourse.bass as bass
import concourse.bacc as bacc
import concourse.tile as tile
from concourse import mybir
from concourse.bass_utils import run_bass_kernel_spmd

F32 = mybir.dt.float32
F32R = mybir.dt.float32r
BF16 = mybir.dt.bfloat16
AF = mybir.ActivationFunctionType

N_CORES = 8
B = 16
BP = B // N_CORES          # batches per core = 2
S = 2048
F = 2048                   # 4H
H = 512
NT = F // 128              # 16 f-tiles
NH = H // 128              # 4 h-tiles
NCHUNK = S // 512          # 4 chunks of 512 per batch


def build_nc():
    nc = bacc.Bacc()
    enc_d = nc.dram_tensor("enc", [BP, S, F], F32, kind="ExternalInput")
    x_d = nc.dram_tensor("x", [BP, F], F32, kind="ExternalInput")
    We_d = nc.dram_tensor("We", [H, F], F32, kind="ExternalInput")
    Wh_d = nc.dram_tensor("Wh", [H, F], F32, kind="ExternalInput")
    Wa_d = nc.dram_tensor("Wa", [H, 2 * H], F32, kind="ExternalInput")
    be_d = nc.dram_tensor("be", [1, H], F32, kind="ExternalInput")
    bh_d = nc.dram_tensor("bh", [1, H], F32, kind="ExternalInput")
    ba_d = nc.dram_tensor("ba", [1, H], F32, kind="ExternalInput")
    v_d = nc.dram_tensor("v", [1, H], F32, kind="ExternalInput")
    id_d = nc.dram_tensor("ident", [128, 128], F32, kind="ExternalInput")
    ctx_d = nc.dram_tensor("ctx_out", [BP, H], F32, kind="ExternalOutput")
    attn_d = nc.dram_tensor("attn_out", [BP, S], F32, kind="ExternalOutput")

    with tile.TileContext(nc) as tc, ExitStack() as ctx:
        pool = ctx.enter_context(tc.tile_pool(name="sb", bufs=1))
        st = ctx.enter_context(tc.tile_pool(name="stage", bufs=2))
        ps = ctx.enter_context(tc.tile_pool(name="ps", bufs=2, space="PSUM"))
        # enc staging pool is created early so weight staging can share its
        # slots (same shapes); after prep the slots recycle to enc tiles.
        enc_pool = ctx.enter_context(tc.tile_pool(name="encp", bufs=2))

        # ---------------- constants / identities ----------------
        id_f32 = pool.tile([128, 128], F32)
        nc.sync.dma_start(out=id_f32, in_=id_d[:, :])
        id_bf = pool.tile([128, 128], BF16)
        nc.gpsimd.dma_start(out=id_bf, in_=id_d[:, :])
        ones1 = pool.tile([1, 128], BF16)
        nc.vector.memset(ones1, 1.0)

        def transpose_128(nat, ncols_tiles, name, host_pool=None, host_tag="",
                          host_bufs=None):
            """nat [128, NH, width]: nat[q,u,c] = W[128u+q, c].
            Returns T_sb [128, ncols_tiles, 512] bf16 with T[p,t,r] = W[r, 128t+p].
            Uses the DMA xbar transpose: out[p, t, r-block u] = in[r, 128t+p]."""
            hp = host_pool if host_pool is not None else pool
            kw = {} if host_bufs is None else {"bufs": host_bufs}
            T_sb = hp.tile([128, ncols_tiles, NH * 128], BF16, name=name,
                           tag=host_tag, **kw)
            for u in range(NH):
                nc.sync.dma_start_transpose(T_sb[:, :, 128 * u:128 * (u + 1)],
                                            nat[:, u, :])
            return T_sb

        def load_weight(dram, width, name):
            nat = enc_pool.tile([128, NH, width], BF16, tag="enc_nat", bufs=2,
                                name=name)
            nc.gpsimd.dma_start(out=nat,
                                in_=dram.rearrange("(u p) f -> p u f", p=128))
            return nat

        def colvec(dram, name, out_dt=F32):
            vn = st.tile([1, H], F32, tag="vecn", bufs=1, name=f"{name}_nat")
            nc.sync.dma_start(out=vn, in_=dram[:, :])
            tp = ps.tile([128, NH], F32, tag="tp", name=f"{name}_ps")
            for m in range(NH):
                nc.tensor.matmul(tp[:, m:m + 1], vn[0:1, 128 * m:128 * (m + 1)],
                                 id_f32[0:1, 0:1], start=True, stop=True)
            out = pool.tile([128, NH], out_dt, name=name)
            nc.vector.tensor_copy(out, tp)
            return out

        enc_pool_ctx = {}

        def stage_chunk(b, chunk):
            s0 = 512 * chunk
            enc_nat = enc_pool.tile([128, 4, F], BF16, tag="enc_nat", bufs=2,
                                    name=f"enc_nat{b}_{chunk}")
            nc.gpsimd.dma_start(
                out=enc_nat,
                in_=enc_d[b, s0:s0 + 512, :].rearrange("(ss p) f -> p ss f", p=128))
            # per-s-tile xbar transposes (HW-proven shape [128, 2048] -> [128,16,128]):
            # encT[p, ss, t, j] = enc[b, s0 + 128*ss + j, 128t + p]
            encT = enc_pool.tile([128, 4, NT, 128], BF16, tag="encT", bufs=3,
                                 name=f"encT{b}_{chunk}")
            for ss in range(4):
                nc.sync.dma_start_transpose(encT[:, ss, :, :], enc_nat[:, ss, :])
            return encT

        def transpose_pe(nat, ncols_tiles, name, host_pool=None, host_tag="",
                         host_bufs=None):
            """Same layout as transpose_128 but via PE matmuls against the
            identity (used during ramp-up while the PE is otherwise idle)."""
            hp = host_pool if host_pool is not None else pool
            kw = {} if host_bufs is None else {"bufs": host_bufs}
            T_sb = hp.tile([128, ncols_tiles, NH * 128], BF16, name=name,
                           tag=host_tag, **kw)
            for t in range(ncols_tiles):
                wtp = ps.tile([128, 512], F32, tag="tp", name=f"{name}_ps{t}")
                for u in range(NH):
                    nc.tensor.matmul(wtp[:, 128 * u:128 * (u + 1)],
                                     nat[:, u, 128 * t:128 * (t + 1)], id_bf,
                                     start=True, stop=True)
                nc.vector.tensor_copy(T_sb[:, t, :], wtp)
            return T_sb

        # ---------------- critical-path-first prep ----------------
        # We + chunk(0,0) gate the first eo-matmuls; Wh/Wa (needed by the
        # first z/tanh) load right behind.  All weight transposes run on the
        # PE, which is idle during ramp-up, keeping the DMA engines free for
        # the enc stream.
        We_nat = load_weight(We_d, F, "We_nat")
        WeT = transpose_pe(We_nat, NT, "WeT")       # WeT[p,t,h] = We[h,128t+p]
        beT = colvec(be_d, "beT")                   # beT[p,m] = be[128m+p]
        enc_pool_ctx[(0, 0)] = stage_chunk(0, 0)
        enc_pool_ctx[(0, 1)] = stage_chunk(0, 1)
        Wh_nat = load_weight(Wh_d, F, "Wh_nat")
        Wa_nat = load_weight(Wa_d, 2 * H, "Wa_nat")
        # WhT is only needed during prep: let it live in an encT-shaped slot
        WhT = transpose_pe(Wh_nat, NT, "WhT", host_pool=enc_pool,
                           host_tag="encT", host_bufs=3)
        WaT = transpose_pe(Wa_nat, 2 * NH, "WaT")   # WaT[p,t,k] = Wa[k,128t+p]
        bhT = colvec(bh_d, "bhT")
        baT = colvec(ba_d, "baT")
        vT = colvec(v_d, "vT", out_dt=F32R)

        # xT [128, NT, BP]
        x_nat = st.tile([BP, F], F32, tag="xnat", bufs=1, name="x_nat")
        nc.sync.dma_start(out=x_nat, in_=x_d[:, :])
        xps = ps.tile([128, NT * BP], F32, tag="tp", name="xps")
        for t in range(NT):
            nc.tensor.matmul(xps[:, BP * t:BP * (t + 1)],
                             x_nat[:, 128 * t:128 * (t + 1)], id_f32[0:BP, 0:BP],
                             start=True, stop=True)
        xT = pool.tile([128, NT, BP], BF16)
        nc.vector.tensor_copy(xT, xps.rearrange("p (t b) -> p t b", b=BP))

        # hT = Wh @ x.T + bh
        hT = pool.tile([128, NH, BP], BF16)
        for k in range(NH):
            hp = ps.tile([128, BP], F32, tag="tp", name=f"h_ps{k}")
            for t in range(NT):
                nc.tensor.matmul(hp, WhT[:, t, 128 * k:128 * (k + 1)], xT[:, t, :],
                                 start=(t == 0), stop=(t == NT - 1))
            nc.scalar.activation(out=hT[:, k, :], in_=hp, func=AF.Identity,
                                 bias=bhT[:, k:k + 1])

        # cT = Wa_h @ h.T + ba
        cT = pool.tile([128, NH, BP], F32)
        for m in range(NH):
            cp = ps.tile([128, BP], F32, tag="tp", name=f"c_ps{m}")
            for k in range(NH):
                nc.tensor.matmul(cp, WaT[:, k, 128 * m:128 * (m + 1)], hT[:, k, :],
                                 start=(k == 0), stop=(k == NH - 1))
            nc.scalar.activation(out=cT[:, m, :], in_=cp, func=AF.Identity,
                                 bias=baT[:, m:m + 1])

        # ---------------- main stream ----------------
        # per-batch tiles so batch b+1 writes don't serialize against batch
        # b's softmax/ctx reads (tile-granular dependency tracking)
        eoTs = [pool.tile([128, NH, S], BF16, name=f"eoT{b}") for b in range(BP)]
        scoress = [pool.tile([1, S], F32, name=f"scores{b}") for b in range(BP)]
        eps = ctx.enter_context(tc.tile_pool(name="eops", bufs=3, space="PSUM"))
        zps = ctx.enter_context(tc.tile_pool(name="zps", bufs=2, space="PSUM"))
        sps = ctx.enter_context(tc.tile_pool(name="scps", bufs=1, space="PSUM"))
        en_pool = ctx.enter_context(tc.tile_pool(name="enrg", bufs=3))

        def emit_eo(b, chunk, encT):
            s0 = 512 * chunk
            for m in range(NH):
                ep = eps.tile([128, 512], F32, tag="eo")
                for t in range(NT):
                    nc.tensor.matmul(ep, WeT[:, t, 128 * m:128 * (m + 1)],
                                     encT[:, :, t, :],
                                     start=(t == 0), stop=(t == NT - 1))
                if m % 2 == 0:
                    nc.scalar.activation(out=eoTs[b][:, m, s0:s0 + 512], in_=ep,
                                         func=AF.Identity, bias=beT[:, m:m + 1])
                else:
                    nc.vector.tensor_scalar_add(eoTs[b][:, m, s0:s0 + 512], ep,
                                                beT[:, m:m + 1])

        def emit_z(b, chunk):
            s0 = 512 * chunk
            sp = sps.tile([1, 512], F32, tag="sc")
            for m2 in range(NH):
                zp = zps.tile([128, 512], F32, tag="z")
                for k in range(NH):
                    nc.tensor.matmul(zp, WaT[:, NH + k, 128 * m2:128 * (m2 + 1)],
                                     eoTs[b][:, k, s0:s0 + 512],
                                     start=(k == 0), stop=(k == NH - 1))
                enrg = en_pool.tile([128, 512], F32R, tag="en")
                nc.scalar.activation(out=enrg, in_=zp, func=AF.Tanh,
                                     bias=cT[:, m2, b:b + 1])
                nc.tensor.matmul(sp, vT[:, m2:m2 + 1], enrg,
                                 start=(m2 == 0), stop=(m2 == NH - 1))
            nc.vector.tensor_copy(scoress[b][0:1, s0:s0 + 512], sp)

        for b in range(BP):
            eoT_b = eoTs[b]
            scores_b = scoress[b]
            for chunk in range(NCHUNK):
                g = b * NCHUNK + chunk
                encT = enc_pool_ctx.pop((b, chunk), None)
                if encT is None:
                    encT = stage_chunk(b, chunk)
                # prefetch up to two chunks ahead of this chunk's compute
                for ng in (g + 1, g + 2):
                    if ng < BP * NCHUNK:
                        nb, nchunk = divmod(ng, NCHUNK)
                        if (nb, nchunk) not in enc_pool_ctx:
                            enc_pool_ctx[(nb, nchunk)] = stage_chunk(nb, nchunk)
                emit_eo(b, chunk, encT)
                # z trails eo by one chunk: priority bias keeps the PE on the
                # eo stream (the DMA consumer) and lets z fill the gaps
                if chunk > 0:
                    emit_z(b, chunk - 1)
            emit_z(b, NCHUNK - 1)

            # ---------------- softmax over S for batch b ----------------
            mx = st.tile([1, 1], F32, tag="sm1", name=f"mx{b}")
            nc.vector.reduce_max(out=mx, in_=scores_b[0:1, :], axis=mybir.AxisListType.X)
            nmx = st.tile([1, 1], F32, tag="sm1", name=f"nmx{b}")
            nc.vector.tensor_scalar_mul(nmx, mx, -1.0)
            ex = st.tile([1, S], F32, tag="smex", bufs=1, name=f"ex{b}")
            nc.scalar.activation(out=ex, in_=scores_b[0:1, :], func=AF.Exp,
                                 bias=nmx[0:1, 0:1])
            # ctx uses UNNORMALIZED exp (scaled by 1/Z at the very end), so
            # the broadcast chain starts before the sum/reciprocal finish
            attn_b = st.tile([1, S], BF16, tag="smbf", bufs=1, name=f"attn_b{b}")
            nc.vector.tensor_copy(attn_b, ex)
            sm = st.tile([1, 1], F32, tag="sm1", name=f"sm{b}")
            nc.vector.reduce_sum(out=sm, in_=ex, axis=mybir.AxisListType.X)
            rs = st.tile([1, 1], F32, tag="sm1", name=f"rs{b}")
            nc.vector.reciprocal(rs, sm)
            nc.vector.tensor_scalar_mul(ex, ex, rs[0:1, 0:1])   # in-place normalize
            nc.sync.dma_start(out=attn_d[b:b + 1, :], in_=ex)
            # broadcast attn over 128 partitions via K=1 ones-matmul
            attn_bc = st.tile([128, S], BF16, tag="attn_bc", bufs=1, name=f"attn_bc{b}")
            for q in range(S // 512):
                bcp = ps.tile([128, 512], F32, tag="tp", name=f"bcp{b}_{q}")
                nc.tensor.matmul(bcp, ones1, attn_b[0:1, 512 * q:512 * (q + 1)],
                                 start=True, stop=True)
                nc.vector.tensor_copy(attn_bc[:, 512 * q:512 * (q + 1)], bcp)

            # ---------------- context ----------------
            cxp = sps.tile([1, H], F32, tag="sc", bufs=1, name=f"cxp{b}")
            for m in range(NH):
                prod = st.tile([128, S], BF16, tag="prod", bufs=2, name=f"prod{b}_{m}")
                ctxT = st.tile([128, 1], F32, tag="ctxT", name=f"ctxT{b}_{m}")
                nc.vector.tensor_mul(prod, eoT_b[:, m, :], attn_bc)
                nc.scalar.activation(out=prod, in_=prod, func=AF.Identity,
                                     accum_out=ctxT)
                # transpose [128,1] -> [1,128] so the DRAM write is contiguous
                nc.tensor.matmul(cxp[0:1, 128 * m:128 * (m + 1)], ctxT, id_f32,
                                 start=True, stop=True)
            ctx_nat = st.tile([1, H], F32, tag="ctxn", name=f"ctxn{b}")
            nc.vector.tensor_scalar_mul(ctx_nat, cxp, rs[0:1, 0:1])  # apply 1/Z
            nc.sync.dma_start(out=ctx_d[b:b + 1, :], in_=ctx_nat)

    nc.finalize()
    return nc


_cache = {}


def _get_nc():
    if "nc" not in _cache:
        _cache["nc"] = build_nc()
    return _cache["nc"]


def make_in_maps(inputs):
    hidden = np.asarray(inputs["hidden"], np.float32)
    enc = np.ascontiguousarray(np.asarray(inputs["encoder_outputs"], np.float32))
    ident = np.eye(128, dtype=np.float32)
    common = dict(
        We=np.asarray(inputs["We"], np.float32),
        Wh=np.asarray(inputs["Wh"], np.float32),
        Wa=np.asarray(inputs["Wa"], np.float32),
        be=np.asarray(inputs["be"], np.float32).reshape(1, H),
        bh=np.asarray(inputs["bh"], np.float32).reshape(1, H),
        ba=np.asarray(inputs["ba"], np.float32).reshape(1, H),
        v=np.asarray(inputs["v"], np.float32).reshape(1, H),
        ident=ident,
    )
    in_maps = []
    for i in range(N_CORES):
        m = dict(common)
        m["enc"] = np.ascontiguousarray(enc[BP * i:BP * (i + 1)])
        m["x"] = np.ascontiguousarray(hidden[-1, BP * i:BP * (i + 1), :])
        in_maps.append(m)
    return in_maps


def run(inputs, **kw):
    nc = _get_nc()
    res = run_bass_kernel_spmd(nc, make_in_maps(inputs), list(range(N_CORES)), **kw)
    ctxs = np.concatenate([r["ctx_out"] for r in res.results], axis=0)
    attns = np.concatenate([r["attn_out"] for r in res.results], axis=0)
    return (ctxs.astype(np.float32), attns.astype(np.float32)), res


def kernel(**inputs):
    out, _ = run(inputs)
    return out
